# revision 19
# baseline (speedup 1.0000x reference)
"""Multi-head attention (B=2, S=2048, E=1024, H=16) on 8 TRN2 NeuronCores.

Sharding: batch x head-group. Core c handles batch b=c//4 and head group
g=c%4 (4 heads = 256 of E). Each core computes its heads' attention output
slice and a partial fc_out product [S, E]; the host sums the 4 partials per
batch and adds b_out.

v2: all-f16/bf16 matmul datapath (f16 runs ~2x f32r per-row on HW), whole
inputs preloaded with host-pretiled contiguous layouts, single fc pass with
K=256 PSUM accumulation (one [S,E] f16 output per core), gpsimd fc drains.

Device-side math per core (pairs pt in {0,1} of 2 heads = 128 dims):
  qpT = (Wq_g @ q[b].T + bq)      [256, S]   f16 (dims on partitions)
  kpT = (Wk_g @ k_c[b].T + bk)    [256, SKV] f16 (k compressed by mask)
  vp  = (v_c[b] @ Wv_g.T + bv)*m  [SKV, 4*65] bf16 (per head: 64 dims + mask col)
  S_T = kpT_h.T-chunks @ qpT_h    [SKV, S] per head (2 heads via tile_position)
  E_T = exp(S_T)  bf16            (no max-subtraction: |energy| <~ 45, safe)
  AV  = vp_aug.T @ E_T -> [65, S]: rows 0-63 unnormalized O_T, row 64 = denom
  O   = AV[0:64] * recip(AV[64])  (recip via [128,16] shuffle + ones-col
                                   broadcast matmul)
  out = O_T.T @ Wo (K=256 accumulated over both pairs in PSUM) -> [S, E] f16

Mask handling is exact: masked K/V rows are removed on the host (gather), so
softmax(where(mask==0, -1e20, e)) == exp(e_valid)/sum(exp(e_valid)).
"""

import os

import ml_dtypes
import numpy as np

B, S, E, H = 2, 2048, 1024, 16
D = E // H           # 64
NCORES = 8
GROUPS = 4           # head groups per batch (cores per batch)
HPG = H // GROUPS    # 4 heads per core
DC = E // GROUPS     # 256 dims per core
NB = E // 128        # 8 contraction chunks over E
QB = 512             # query block
NQB = S // QB        # 4

_CACHE = {}


def _split_excess_waits(nc, max_waits=1):
    """walrus rejects instructions carrying >1 sem wait; spread extras onto
    single-wait NoOps inserted before the instruction on the same engine."""
    import concourse.mybir as mybir

    n_split = 0
    for f in nc.m.functions:
        for bb in f.blocks:
            out, changed = [], False
            for ins in bb.instructions:
                si = ins.sync_info
                if si is not None and si.on_wait is not None and len(si.on_wait) > max_waits:
                    waits = list(si.on_wait)
                    for w in waits[:-max_waits]:
                        out.append(mybir.InstNoOp(
                            name=nc.get_next_instruction_name(),
                            engine=ins.engine, ins=[], outs=[],
                            sync_info=mybir.SyncInfo(on_wait=[w], on_update=[])))
                        n_split += 1
                    ins.sync_info = mybir.SyncInfo(
                        on_wait=waits[-max_waits:], on_update=list(si.on_update))
                    changed = True
                out.append(ins)
            if changed:
                bb.instructions = out
    return n_split


def _build(skv, split_waits=True):
    import concourse.bass as bass
    import concourse.mybir as mybir
    import concourse.tile as tile

    f32 = mybir.dt.float32
    f32r = mybir.dt.float32r
    f16 = mybir.dt.float16
    bf16 = mybir.dt.bfloat16
    Alu = mybir.AluOpType
    Act = mybir.ActivationFunctionType

    nsk = skv // 128     # 128-wide kv chunks
    ncp = nsk // 2       # exp chunk pairs (1024-wide exp tiles)
    kblocks = [512] * (skv // 512) + ([skv % 512] if skv % 512 else [])
    nkb = len(kblocks)   # xk stored as nkb padded 512-blocks

    nc = bass.Bass()
    xq_d = nc.declare_dram_parameter("xq", [128, NQB, NB, QB], f16, isOutput=False)
    xk_d = nc.declare_dram_parameter("xk", [128, nkb, NB, 512], f16, isOutput=False)
    xv_d = nc.declare_dram_parameter("xv", [128, nsk, NB, 128], f16, isOutput=False)
    wq_d = nc.declare_dram_parameter("wq", [128, NB, DC], f16, isOutput=False)
    wk_d = nc.declare_dram_parameter("wk", [128, NB, DC], f16, isOutput=False)
    wv_d = nc.declare_dram_parameter("wv", [128, NB, DC], f16, isOutput=False)
    wo_d = nc.declare_dram_parameter("wo", [128, 2, E], f16, isOutput=False)
    bq_d = nc.declare_dram_parameter("bq", [128, 2], f32, isOutput=False)
    bk_d = nc.declare_dram_parameter("bk", [128, 2], f32, isOutput=False)
    bv_d = nc.declare_dram_parameter("bv", [DC], f32, isOutput=False)
    vm_d = nc.declare_dram_parameter("vmask", [128, nsk], f32, isOutput=False)
    ones_d = nc.declare_dram_parameter("ones64", [1, 64], bf16, isOutput=False)
    out_d = nc.declare_dram_parameter("out", [S, E], f16, isOutput=True)

    with tile.TileContext(nc) as tc:
        with (
            tc.tile_pool(name="weights", bufs=1) as wpool,
            tc.tile_pool(name="xin", bufs=1) as xpool,
            tc.tile_pool(name="consts", bufs=1) as cpool,
            tc.tile_pool(name="persist", bufs=1) as ppool,
            tc.tile_pool(name="small", bufs=2) as smpool,
            tc.tile_pool(name="et", bufs=2) as etpool,
            tc.tile_pool(name="oun", bufs=4) as opool,
            tc.tile_pool(name="nrm", bufs=4) as nrpool,
            tc.tile_pool(name="outp", bufs=4) as obpool,
            tc.tile_pool(name="mm_ps", bufs=3, space="PSUM") as mmps,
            tc.tile_pool(name="av_ps", bufs=2, space="PSUM") as avps,
        ):
            # ---- weights / x / constants (k path first: it gates scores) ----
            wk_t = wpool.tile([128, NB, DC], f16, tag="wk")
            wq_t = wpool.tile([128, NB, DC], f16, tag="wq")
            wv_t = wpool.tile([128, NB, DC], f16, tag="wv")
            wo_t = wpool.tile([128, 2, E], f16, tag="wo")
            xq_t = xpool.tile([128, NQB, NB, QB], f16, tag="xq")
            xk_t = xpool.tile([128, nkb, NB, 512], f16, tag="xk")
            xv_t = xpool.tile([128, nsk, NB, 128], f16, tag="xv")
            bq_t = cpool.tile([128, 2], f32, tag="bq")
            bk_t = cpool.tile([128, 2], f32, tag="bk")
            bv_t = cpool.tile([128, DC], f32, tag="bv")
            vm_t = cpool.tile([128, nsk], f32, tag="vm")
            ones_t = cpool.tile([1, 64], bf16, tag="ones")

            def flat(ap):
                return ap.rearrange("p ... -> p (...)")

            nc.sync.dma_start(flat(wk_t[:]), flat(wk_d[:]))
            nc.sync.dma_start(bk_t[:], bk_d[:])
            for kb in range(nkb):
                nc.sync.dma_start(flat(xk_t[:, kb]), flat(xk_d[:, kb]))
            nc.sync.dma_start(flat(wq_t[:]), flat(wq_d[:]))
            nc.sync.dma_start(bq_t[:], bq_d[:])
            nc.sync.dma_start(flat(xq_t[:, 0]), flat(xq_d[:, 0]))
            nc.sync.dma_start(flat(wv_t[:]), flat(wv_d[:]))
            nc.sync.dma_start(bv_t[:], bv_d[None, :].to_broadcast((128, DC)))
            nc.sync.dma_start(vm_t[:], vm_d[:])
            nc.sync.dma_start(ones_t[:], ones_d[:])
            for h in range(2):
                nc.sync.dma_start(flat(xv_t[:, h * (nsk // 2):(h + 1) * (nsk // 2)]),
                                  flat(xv_d[:, h * (nsk // 2):(h + 1) * (nsk // 2)]))
            for nb in range(1, NQB):
                nc.sync.dma_start(flat(xq_t[:, nb]), flat(xq_d[:, nb]))
            nc.sync.dma_start(flat(wo_t[:]), flat(wo_d[:]))

            # ---- persistent activations ----
            qpT = ppool.tile([128, 2, S], f16, tag="qpT")
            kpT = ppool.tile([128, 2, skv], f16, tag="kpT")
            vp = ppool.tile([128, nsk, HPG * (D + 1)], bf16, tag="vp")
            o_f16 = ppool.tile([128, 2, S], f16, tag="o_f16")

            def proj_k(kb, w):
                for mc in range(2):
                    ps = mmps.tile([128, 1024], f32, tag="mm", name="kp_ps")
                    for kc in range(NB):
                        nc.tensor.matmul(
                            ps[:, 0:w], wk_t[:, kc, mc * 128:(mc + 1) * 128],
                            xk_t[:, kb, kc, 0:w],
                            start=(kc == 0), stop=(kc == NB - 1))
                    nc.vector.tensor_tensor(
                        out=kpT[:, mc, kb * 512:kb * 512 + w], in0=ps[:, 0:w],
                        in1=bk_t[:, mc:mc + 1].to_broadcast((128, w)), op=Alu.add)

            def proj_q(nb):
                for mc in range(2):
                    ps = mmps.tile([128, 1024], f32, tag="mm", name="qp_ps")
                    for kc in range(NB):
                        nc.tensor.matmul(
                            ps[:, 0:512], wq_t[:, kc, mc * 128:(mc + 1) * 128],
                            xq_t[:, nb, kc, :], start=(kc == 0), stop=(kc == NB - 1))
                    nc.vector.tensor_tensor(
                        out=qpT[:, mc, nb * 512:(nb + 1) * 512], in0=ps[:, 0:512],
                        in1=bq_t[:, mc:mc + 1].to_broadcast((128, 512)), op=Alu.add)

            def proj_v(sc):
                ps = mmps.tile([128, 1024], f32, tag="mm", name="vp_ps")
                for kc in range(NB):
                    nc.tensor.matmul(
                        ps[:, 0:DC], xv_t[:, sc, kc, :], wv_t[:, kc, :],
                        start=(kc == 0), stop=(kc == NB - 1))
                t1 = smpool.tile([128, DC], f32, tag="vtmp")
                nc.vector.tensor_tensor(out=t1[:], in0=ps[:, 0:DC], in1=bv_t[:],
                                        op=Alu.add)
                vps = vp[:, sc, :].rearrange("p (h w) -> p h w", w=D + 1)
                nc.vector.tensor_tensor(
                    out=vps[:, :, 0:D],
                    in0=t1.rearrange("p (h w) -> p h w", w=D),
                    in1=vm_t[:, sc:sc + 1, None].to_broadcast((128, HPG, D)),
                    op=Alu.mult)
                nc.vector.tensor_copy(
                    out=vps[:, :, D:D + 1],
                    in_=vm_t[:, sc:sc + 1, None].to_broadcast((128, HPG, 1)))

            # lead-in: kpT fully, first qpT block, first two vp chunks
            for kb, w in enumerate(kblocks):
                proj_k(kb, w)
            proj_q(0)
            proj_v(0)
            proj_v(1)

            o_un = {}    # (pt, j) -> [65, S] bf16 unnormalized O_T + denom row
            rcr = {}     # (pt, j) -> [1, S] f16 reciprocal denominators

            def scores(pt, qb, j, cp):
                """2 score matmuls (skc=2cp, 2cp+1) + 1024-wide exp."""
                psx = mmps.tile([128, 1024], f32, tag="mm", name=f"sc{j}")
                for h in range(2):
                    skc = 2 * cp + h
                    nc.tensor.matmul(
                        psx[:, h * 512:(h + 1) * 512],
                        kpT[64 * j:64 * j + 64, pt, skc * 128:(skc + 1) * 128],
                        qpT[64 * j:64 * j + 64, pt, qb * QB:(qb + 1) * QB],
                        start=True, stop=True, tile_position=(64 * j, 0))
                return psx

            def expv(psx, et, cp):
                nc.scalar.activation(
                    et[:, 2 * cp:2 * cp + 2, :].rearrange("p a b -> p (a b)"),
                    psx[:], Act.Exp)

            def av(pt, qb, j, cp, av_ps, et):
                hl = 2 * pt + j
                for h in range(2):
                    skc = 2 * cp + h
                    nc.tensor.matmul(
                        av_ps[:], vp[:, skc, hl * (D + 1):(hl + 1) * (D + 1)],
                        et[:, skc, :], start=(skc == 0), stop=(skc == nsk - 1))

            def norm_chain_pre(pt, j, qb):
                """recip of one qb's denominators: [1,QB] -> [128,4] shuffle,
                recip, cast bf16, shuffle back. Latency hides under compute."""
                s128 = nrpool.tile([128, QB // 128], bf16, tag="s128")
                nc.sync.dma_start(s128[:], o_un[pt, j][64:65, qb * QB:(qb + 1) * QB])
                r32 = nrpool.tile([128, QB // 128], f32, tag="r32")
                nc.vector.reciprocal(out=r32[:], in_=s128[:])
                rr = nrpool.tile([128, QB // 128], bf16, tag="r16")
                nc.vector.tensor_copy(out=rr[:], in_=r32[:])
                nc.sync.dma_start(rcr[pt, j][0:1, qb * QB:(qb + 1) * QB], rr[:])

            def norm_mult(pt, j, qb):
                """o_f16 = o_un * bcast(recip) for one qb slice (PE+DVE)."""
                rc_ps = mmps.tile([128, 1024], f32, tag="mm", name="rc_ps")
                nc.tensor.matmul(
                    rc_ps[0:64, 0:512], ones_t[:],
                    rcr[pt, j][0:1, qb * QB:(qb + 1) * QB], start=True, stop=True)
                nc.vector.tensor_tensor(
                    out=o_f16[64 * j:64 * j + 64, pt, qb * QB:(qb + 1) * QB],
                    in0=o_un[pt, j][0:64, qb * QB:(qb + 1) * QB],
                    in1=rc_ps[0:64, 0:512], op=Alu.mult)

            def fc(sqc, drain="alt"):
                ps = mmps.tile([128, 1024], f32, tag="mm", name="fc")
                for eb in range(2):
                    for pt in range(2):
                        nc.tensor.matmul(
                            ps[:, eb * 512:(eb + 1) * 512],
                            o_f16[:, pt, sqc * 128:(sqc + 1) * 128],
                            wo_t[:, pt, eb * 512:(eb + 1) * 512],
                            start=(pt == 0), stop=(pt == 1))
                ob = obpool.tile([128, 1024], f16, tag="ob")
                if drain == "v" or (drain == "alt" and sqc % 2 == 1):
                    nc.vector.tensor_copy(out=ob[:], in_=ps[:])
                else:
                    nc.scalar.activation(ob[:], ps[:], Act.Copy)
                nc.sync.dma_start(out_d[sqc * 128:(sqc + 1) * 128, :], ob[:])

            # ---- attention: pt pairs x query blocks, hand-interleaved ----
            for pt in range(2):
                for j in range(2):
                    o_un[pt, j] = opool.tile([65, S], bf16, tag="oun",
                                             name=f"o_un{pt}{j}")
                    rcr[pt, j] = nrpool.tile([1, S], bf16, tag="rcr",
                                             name=f"rcr{pt}{j}")
                for qb in range(NQB):
                    # fillers executed early in this qb's PE stream
                    fillers = []
                    if pt == 0 and qb == 0:
                        fillers = [lambda sc=sc: proj_v(sc) for sc in range(2, nsk)]
                    if pt == 1:
                        # pt0 normalization + prev-qb pt1 norm/fc interleave
                        fillers = [lambda j=j, qb=qb: norm_mult(0, j, qb)
                                   for j in range(2)]
                        if qb >= 1:
                            fillers += [lambda j=j, q=qb - 1: norm_mult(1, j, q)
                                        for j in range(2)]
                            fillers += [lambda s=s: fc(s, drain="v")
                                        for s in range((qb - 1) * 4, qb * 4)]

                    et = {j: etpool.tile([128, nsk, 512], bf16, tag="et",
                                         name=f"et{j}") for j in range(2)}
                    avp = {j: avps.tile([65, 512], f32, tag="av", name=f"av{j}")
                           for j in range(2)}
                    nf = 0
                    for cp in range(ncp):
                        for j in range(2):
                            psx = scores(pt, qb, j, cp)
                            expv(psx, et[j], cp)
                            if cp > 0:
                                av(pt, qb, j, cp - 1, avp[j], et[j])
                                if nf < len(fillers):
                                    fillers[nf]()
                                    nf += 1
                    for j in range(2):
                        av(pt, qb, j, ncp - 1, avp[j], et[j])
                    while nf < len(fillers):
                        fillers[nf]()
                        nf += 1
                    # drain unnormalized AV + denominator row
                    for j in range(2):
                        nc.vector.tensor_copy(
                            out=o_un[pt, j][:, qb * QB:(qb + 1) * QB],
                            in_=avp[j][:])
                        norm_chain_pre(pt, j, qb)
                    # trailing qpT projection blocks overlap attention
                    if pt == 0 and qb + 1 < NQB:
                        proj_q(qb + 1)

            # ---- tail: last block's pt1 normalization + fc ----
            for j in range(2):
                norm_mult(1, j, NQB - 1)
            for sqc in range((NQB - 1) * 4, NQB * 4):
                fc(sqc)

    if split_waits:
        _split_excess_waits(nc)
    return nc


def _prep_inputs(q, k, v, mask, W_qkv, b_qkv, W_out, b_out):
    """Host-side shard/layout prep. Returns (skv, in_maps)."""
    q = np.asarray(q, dtype=np.float32)
    k = np.asarray(k, dtype=np.float32)
    v = np.asarray(v, dtype=np.float32)
    mask = np.asarray(mask)
    W_qkv = np.asarray(W_qkv, dtype=np.float32)
    b_qkv = np.asarray(b_qkv, dtype=np.float32)
    W_out = np.asarray(W_out, dtype=np.float32)

    valid = [np.nonzero(mask[b, 0, 0] != 0)[0] for b in range(B)]
    cnts = [len(vi) for vi in valid]
    skv = max(512, max((c + 255) // 256 * 256 for c in cnts))
    nsk = skv // 128

    def tile_x(xT, inner):
        # [E, S'] -> [128, S'/ (NB*inner) ... ] partition-major contiguous
        n_out = xT.shape[1] // inner
        return np.ascontiguousarray(
            xT.reshape(NB, 128, n_out, inner).transpose(1, 2, 0, 3)
        ).astype(np.float16)

    qT, kc_t, vc_t, vms = [], [], [], []
    for b in range(B):
        qT.append(tile_x(np.ascontiguousarray(q[b].T), QB))
        kt = np.zeros((E, skv), np.float32)
        vt = np.zeros((E, skv), np.float32)
        kt[:, :cnts[b]] = k[b][valid[b]].T
        vt[:, :cnts[b]] = v[b][valid[b]].T
        # xk: [128, nkb, NB, 512] padded 512-blocks, contiguous per (p, kb)
        nkb = (skv + 511) // 512
        ktp = np.zeros((E, nkb * 512), np.float32)
        ktp[:, :skv] = kt
        kc_t.append(np.ascontiguousarray(
            ktp.reshape(NB, 128, nkb, 512).transpose(1, 2, 0, 3)
        ).astype(np.float16))
        vc_t.append(tile_x(vt, 128))
        vm = np.zeros((skv,), np.float32)
        vm[:cnts[b]] = 1.0
        vms.append(np.ascontiguousarray(vm.reshape(nsk, 128).T))

    def tile_w(wT):
        # [E, DC] -> [128, NB, DC]
        return np.ascontiguousarray(
            wT.reshape(NB, 128, DC).transpose(1, 0, 2)).astype(np.float16)

    in_maps = []
    for c in range(NCORES):
        b, g = divmod(c, GROUPS)
        sl = slice(g * DC, (g + 1) * DC)
        in_maps.append({
            "xq": qT[b], "xk": kc_t[b], "xv": vc_t[b],
            "wq": tile_w(np.ascontiguousarray(W_qkv[sl, :].T)),
            "wk": tile_w(np.ascontiguousarray(W_qkv[E:][sl, :].T)),
            "wv": tile_w(np.ascontiguousarray(W_qkv[2 * E:][sl, :].T)),
            "wo": np.ascontiguousarray(
                W_out[:, sl].T.reshape(2, 128, E).transpose(1, 0, 2)
            ).astype(np.float16),
            "bq": np.ascontiguousarray(b_qkv[sl].reshape(2, 128).T),
            "bk": np.ascontiguousarray(b_qkv[E:][sl].reshape(2, 128).T),
            "bv": np.ascontiguousarray(b_qkv[2 * E:][sl]),
            "vmask": vms[b],
            "ones64": np.ones((1, 64)).astype(ml_dtypes.bfloat16),
        })
    return skv, in_maps


def kernel(q, k, v, mask, W_qkv, b_qkv, W_out, b_out):
    from concourse import bass_utils

    skv, in_maps = _prep_inputs(q, k, v, mask, W_qkv, b_qkv, W_out, b_out)
    if skv not in _CACHE:
        _CACHE[skv] = _build(skv)
    nc = _CACHE[skv]

    trace = os.environ.get("KERNEL_TRACE") == "1"
    if trace:
        bass_utils.upload_artifacts = lambda tmpdir: "local://" + tmpdir
    res = bass_utils.run_bass_kernel_spmd(
        nc, in_maps, list(range(NCORES)), trace=trace)
    if trace:
        print(f"HW exec time: {res.exec_time_ns} ns")

    b_out = np.asarray(b_out, dtype=np.float32)
    out = np.zeros((B, S, E), np.float32)
    for c in range(NCORES):
        out[c // GROUPS] += res.results[c]["out"].astype(np.float32)
    out += b_out[None, None, :]
    return out


# revision 20
# speedup vs baseline: 1.0658x; 1.0658x over previous
"""Multi-head attention (B=2, S=2048, E=1024, H=16) on 8 TRN2 NeuronCores.

Sharding: batch x head-group. Core c handles batch b=c//4 and head group
g=c%4 (4 heads = 256 of E). Each core computes its heads' attention output
slice and a partial fc_out product [S, E]; the host sums the 4 partials per
batch and adds b_out.

v2: all-f16/bf16 matmul datapath (f16 runs ~2x f32r per-row on HW), whole
inputs preloaded with host-pretiled contiguous layouts, single fc pass with
K=256 PSUM accumulation (one [S,E] f16 output per core), gpsimd fc drains.

Device-side math per core (pairs pt in {0,1} of 2 heads = 128 dims):
  qpT = (Wq_g @ q[b].T + bq)      [256, S]   f16 (dims on partitions)
  kpT = (Wk_g @ k_c[b].T + bk)    [256, SKV] f16 (k compressed by mask)
  vp  = (v_c[b] @ Wv_g.T + bv)*m  [SKV, 4*65] bf16 (per head: 64 dims + mask col)
  S_T = kpT_h.T-chunks @ qpT_h    [SKV, S] per head (2 heads via tile_position)
  E_T = exp(S_T)  bf16            (no max-subtraction: |energy| <~ 45, safe)
  AV  = vp_aug.T @ E_T -> [65, S]: rows 0-63 unnormalized O_T, row 64 = denom
  O   = AV[0:64] * recip(AV[64])  (recip via [128,16] shuffle + ones-col
                                   broadcast matmul)
  out = O_T.T @ Wo (K=256 accumulated over both pairs in PSUM) -> [S, E] f16

Mask handling is exact: masked K/V rows are removed on the host (gather), so
softmax(where(mask==0, -1e20, e)) == exp(e_valid)/sum(exp(e_valid)).
"""

import os

import ml_dtypes
import numpy as np

B, S, E, H = 2, 2048, 1024, 16
D = E // H           # 64
NCORES = 8
GROUPS = 4           # head groups per batch (cores per batch)
HPG = H // GROUPS    # 4 heads per core
DC = E // GROUPS     # 256 dims per core
NB = E // 128        # 8 contraction chunks over E
QB = 512             # query block
NQB = S // QB        # 4

_CACHE = {}


def _split_excess_waits(nc, max_waits=1):
    """walrus rejects instructions carrying >1 sem wait; spread extras onto
    single-wait NoOps inserted before the instruction on the same engine."""
    import concourse.mybir as mybir

    n_split = 0
    for f in nc.m.functions:
        for bb in f.blocks:
            out, changed = [], False
            for ins in bb.instructions:
                si = ins.sync_info
                if si is not None and si.on_wait is not None and len(si.on_wait) > max_waits:
                    waits = list(si.on_wait)
                    for w in waits[:-max_waits]:
                        out.append(mybir.InstNoOp(
                            name=nc.get_next_instruction_name(),
                            engine=ins.engine, ins=[], outs=[],
                            sync_info=mybir.SyncInfo(on_wait=[w], on_update=[])))
                        n_split += 1
                    ins.sync_info = mybir.SyncInfo(
                        on_wait=waits[-max_waits:], on_update=list(si.on_update))
                    changed = True
                out.append(ins)
            if changed:
                bb.instructions = out
    return n_split


def _build(skv, split_waits=True):
    import concourse.bass as bass
    import concourse.mybir as mybir
    import concourse.tile as tile

    f32 = mybir.dt.float32
    f32r = mybir.dt.float32r
    f16 = mybir.dt.float16
    bf16 = mybir.dt.bfloat16
    Alu = mybir.AluOpType
    Act = mybir.ActivationFunctionType

    nsk = skv // 128     # 128-wide kv chunks
    ncp = nsk // 2       # exp chunk pairs (1024-wide exp tiles)
    kblocks = [512] * (skv // 512) + ([skv % 512] if skv % 512 else [])
    nkb = len(kblocks)   # xk stored as nkb padded 512-blocks

    nc = bass.Bass()
    xq_d = nc.declare_dram_parameter("xq", [128, NQB, NB, QB], f16, isOutput=False)
    xk_d = nc.declare_dram_parameter("xk", [128, nkb, NB, 512], f16, isOutput=False)
    xv_d = nc.declare_dram_parameter("xv", [128, nsk, NB, 128], f16, isOutput=False)
    wq_d = nc.declare_dram_parameter("wq", [128, NB, DC], f16, isOutput=False)
    wk_d = nc.declare_dram_parameter("wk", [128, NB, DC], f16, isOutput=False)
    wv_d = nc.declare_dram_parameter("wv", [128, NB, DC], f16, isOutput=False)
    wo_d = nc.declare_dram_parameter("wo", [128, 2, E], f16, isOutput=False)
    bq_d = nc.declare_dram_parameter("bq", [128, 2], f32, isOutput=False)
    bk_d = nc.declare_dram_parameter("bk", [128, 2], f32, isOutput=False)
    bv_d = nc.declare_dram_parameter("bv", [DC], f32, isOutput=False)
    vm_d = nc.declare_dram_parameter("vmask", [128, nsk], f32, isOutput=False)
    ones_d = nc.declare_dram_parameter("ones64", [1, 64], bf16, isOutput=False)
    out_d = nc.declare_dram_parameter("out", [S, E], f16, isOutput=True)

    with tile.TileContext(nc) as tc:
        with (
            tc.tile_pool(name="weights", bufs=1) as wpool,
            tc.tile_pool(name="xin", bufs=1) as xpool,
            tc.tile_pool(name="consts", bufs=1) as cpool,
            tc.tile_pool(name="persist", bufs=1) as ppool,
            tc.tile_pool(name="small", bufs=2) as smpool,
            tc.tile_pool(name="et", bufs=2) as etpool,
            tc.tile_pool(name="oun", bufs=4) as opool,
            tc.tile_pool(name="nrm", bufs=4) as nrpool,
            tc.tile_pool(name="outp", bufs=4) as obpool,
            tc.tile_pool(name="mm_ps", bufs=3, space="PSUM") as mmps,
            tc.tile_pool(name="av_ps", bufs=2, space="PSUM") as avps,
        ):
            # ---- weights / x / constants (k path first: it gates scores) ----
            wk_t = wpool.tile([128, NB, DC], f16, tag="wk")
            wq_t = wpool.tile([128, NB, DC], f16, tag="wq")
            wv_t = wpool.tile([128, NB, DC], f16, tag="wv")
            wo_t = wpool.tile([128, 2, E], f16, tag="wo")
            xq_t = xpool.tile([128, NQB, NB, QB], f16, tag="xq")
            xk_t = xpool.tile([128, nkb, NB, 512], f16, tag="xk")
            xv_t = xpool.tile([128, nsk, NB, 128], f16, tag="xv")
            bq_t = cpool.tile([128, 2], f32, tag="bq")
            bk_t = cpool.tile([128, 2], f32, tag="bk")
            bv_t = cpool.tile([128, DC], f32, tag="bv")
            vm_t = cpool.tile([128, nsk], f32, tag="vm")
            ones_t = cpool.tile([1, 64], bf16, tag="ones")

            def flat(ap):
                return ap.rearrange("p ... -> p (...)")

            nc.sync.dma_start(flat(wk_t[:]), flat(wk_d[:]))
            nc.sync.dma_start(bk_t[:], bk_d[:])
            for kb in range(nkb):
                nc.sync.dma_start(flat(xk_t[:, kb]), flat(xk_d[:, kb]))
            nc.sync.dma_start(flat(wq_t[:]), flat(wq_d[:]))
            nc.sync.dma_start(bq_t[:], bq_d[:])
            nc.sync.dma_start(flat(xq_t[:, 0]), flat(xq_d[:, 0]))
            nc.sync.dma_start(flat(wv_t[:]), flat(wv_d[:]))
            nc.sync.dma_start(bv_t[:], bv_d[None, :].to_broadcast((128, DC)))
            nc.sync.dma_start(vm_t[:], vm_d[:])
            nc.sync.dma_start(ones_t[:], ones_d[:])
            for h in range(2):
                nc.sync.dma_start(flat(xv_t[:, h * (nsk // 2):(h + 1) * (nsk // 2)]),
                                  flat(xv_d[:, h * (nsk // 2):(h + 1) * (nsk // 2)]))
            for nb in range(1, NQB):
                nc.sync.dma_start(flat(xq_t[:, nb]), flat(xq_d[:, nb]))
            nc.sync.dma_start(flat(wo_t[:]), flat(wo_d[:]))

            # ---- persistent activations ----
            qpT = ppool.tile([128, 2, S], f16, tag="qpT")
            kpT = ppool.tile([128, 2, skv], f16, tag="kpT")
            vp = ppool.tile([128, nsk, HPG * (D + 1)], bf16, tag="vp")
            o_f16 = ppool.tile([128, 2, S], f16, tag="o_f16")

            def proj_k(kb, w):
                for mc in range(2):
                    ps = mmps.tile([128, 1024], f32, tag="mm", name="kp_ps")
                    for kc in range(NB):
                        nc.tensor.matmul(
                            ps[:, 0:w], wk_t[:, kc, mc * 128:(mc + 1) * 128],
                            xk_t[:, kb, kc, 0:w],
                            start=(kc == 0), stop=(kc == NB - 1))
                    nc.vector.tensor_tensor(
                        out=kpT[:, mc, kb * 512:kb * 512 + w], in0=ps[:, 0:w],
                        in1=bk_t[:, mc:mc + 1].to_broadcast((128, w)), op=Alu.add)

            def proj_q(nb):
                for mc in range(2):
                    ps = mmps.tile([128, 1024], f32, tag="mm", name="qp_ps")
                    for kc in range(NB):
                        nc.tensor.matmul(
                            ps[:, 0:512], wq_t[:, kc, mc * 128:(mc + 1) * 128],
                            xq_t[:, nb, kc, :], start=(kc == 0), stop=(kc == NB - 1))
                    nc.vector.tensor_tensor(
                        out=qpT[:, mc, nb * 512:(nb + 1) * 512], in0=ps[:, 0:512],
                        in1=bq_t[:, mc:mc + 1].to_broadcast((128, 512)), op=Alu.add)

            def proj_v(sc):
                ps = mmps.tile([128, 1024], f32, tag="mm", name="vp_ps")
                for kc in range(NB):
                    nc.tensor.matmul(
                        ps[:, 0:DC], xv_t[:, sc, kc, :], wv_t[:, kc, :],
                        start=(kc == 0), stop=(kc == NB - 1))
                t1 = smpool.tile([128, DC], f32, tag="vtmp")
                nc.vector.tensor_tensor(out=t1[:], in0=ps[:, 0:DC], in1=bv_t[:],
                                        op=Alu.add)
                vps = vp[:, sc, :].rearrange("p (h w) -> p h w", w=D + 1)
                nc.vector.tensor_tensor(
                    out=vps[:, :, 0:D],
                    in0=t1.rearrange("p (h w) -> p h w", w=D),
                    in1=vm_t[:, sc:sc + 1, None].to_broadcast((128, HPG, D)),
                    op=Alu.mult)
                nc.vector.tensor_copy(
                    out=vps[:, :, D:D + 1],
                    in_=vm_t[:, sc:sc + 1, None].to_broadcast((128, HPG, 1)))

            # lead-in: kpT fully, first qpT block, first two vp chunks
            for kb, w in enumerate(kblocks):
                proj_k(kb, w)
            proj_q(0)
            proj_v(0)
            proj_v(1)

            o_un = {}    # (pt, j) -> [65, S] bf16 unnormalized O_T + denom row
            rcr = {}     # (pt, j) -> [1, S] f16 reciprocal denominators

            def scores(pt, qb, j, cp):
                """2 score matmuls (skc=2cp, 2cp+1) + 1024-wide exp."""
                psx = mmps.tile([128, 1024], f32, tag="mm", name=f"sc{j}")
                for h in range(2):
                    skc = 2 * cp + h
                    nc.tensor.matmul(
                        psx[:, h * 512:(h + 1) * 512],
                        kpT[64 * j:64 * j + 64, pt, skc * 128:(skc + 1) * 128],
                        qpT[64 * j:64 * j + 64, pt, qb * QB:(qb + 1) * QB],
                        start=True, stop=True, tile_position=(64 * j, 0))
                return psx

            def expv(psx, et, cp):
                nc.scalar.activation(
                    et[:, 2 * cp:2 * cp + 2, :].rearrange("p a b -> p (a b)"),
                    psx[:], Act.Exp)

            def av(pt, qb, j, cp, av_ps, et):
                hl = 2 * pt + j
                for h in range(2):
                    skc = 2 * cp + h
                    nc.tensor.matmul(
                        av_ps[:], vp[:, skc, hl * (D + 1):(hl + 1) * (D + 1)],
                        et[:, skc, :], start=(skc == 0), stop=(skc == nsk - 1))

            def norm_chain_pre(pt, j, qb):
                """recip of one qb's denominators: [1,QB] -> [128,4] shuffle,
                recip, cast bf16, shuffle back. Latency hides under compute."""
                s128 = nrpool.tile([128, QB // 128], bf16, tag="s128")
                nc.sync.dma_start(s128[:], o_un[pt, j][64:65, qb * QB:(qb + 1) * QB])
                r32 = nrpool.tile([128, QB // 128], f32, tag="r32")
                nc.vector.reciprocal(out=r32[:], in_=s128[:])
                rr = nrpool.tile([128, QB // 128], bf16, tag="r16")
                nc.vector.tensor_copy(out=rr[:], in_=r32[:])
                nc.sync.dma_start(rcr[pt, j][0:1, qb * QB:(qb + 1) * QB], rr[:])

            def norm_mult(pt, j, qb):
                """o_f16 = o_un * bcast(recip) for one qb slice (PE+DVE)."""
                rc_ps = mmps.tile([128, 1024], f32, tag="mm", name="rc_ps")
                nc.tensor.matmul(
                    rc_ps[0:64, 0:512], ones_t[:],
                    rcr[pt, j][0:1, qb * QB:(qb + 1) * QB], start=True, stop=True)
                nc.vector.tensor_tensor(
                    out=o_f16[64 * j:64 * j + 64, pt, qb * QB:(qb + 1) * QB],
                    in0=o_un[pt, j][0:64, qb * QB:(qb + 1) * QB],
                    in1=rc_ps[0:64, 0:512], op=Alu.mult)

            def fc(sqc, drain="alt"):
                ps = mmps.tile([128, 1024], f32, tag="mm", name="fc")
                for eb in range(2):
                    for pt in range(2):
                        nc.tensor.matmul(
                            ps[:, eb * 512:(eb + 1) * 512],
                            o_f16[:, pt, sqc * 128:(sqc + 1) * 128],
                            wo_t[:, pt, eb * 512:(eb + 1) * 512],
                            start=(pt == 0), stop=(pt == 1))
                ob = obpool.tile([128, 1024], f16, tag="ob")
                if drain == "v" or (drain == "alt" and sqc % 2 == 1):
                    nc.vector.tensor_copy(out=ob[:], in_=ps[:])
                else:
                    nc.scalar.activation(ob[:], ps[:], Act.Copy)
                nc.sync.dma_start(out_d[sqc * 128:(sqc + 1) * 128, :], ob[:])

            # ---- attention: pt pairs x query blocks, hand-interleaved ----
            for pt in range(2):
                for j in range(2):
                    o_un[pt, j] = opool.tile([65, S], bf16, tag="oun",
                                             name=f"o_un{pt}{j}")
                    rcr[pt, j] = nrpool.tile([1, S], bf16, tag="rcr",
                                             name=f"rcr{pt}{j}")
                for qb in range(NQB):
                    # fillers executed early in this qb's PE stream
                    fillers = []
                    if pt == 0 and qb == 0:
                        fillers = [lambda sc=sc: proj_v(sc) for sc in range(2, nsk)]
                    if pt == 1:
                        # pt0 normalization + prev-qb pt1 normalization
                        fillers = [lambda j=j, qb=qb: norm_mult(0, j, qb)
                                   for j in range(2)]
                        if qb >= 1:
                            fillers += [lambda j=j, q=qb - 1: norm_mult(1, j, q)
                                        for j in range(2)]

                    et = {j: etpool.tile([128, nsk, 512], bf16, tag="et",
                                         name=f"et{j}") for j in range(2)}
                    avp = {j: avps.tile([65, 512], f32, tag="av", name=f"av{j}")
                           for j in range(2)}
                    nf = 0
                    for cp in range(ncp):
                        for j in range(2):
                            psx = scores(pt, qb, j, cp)
                            expv(psx, et[j], cp)
                            if cp > 0:
                                av(pt, qb, j, cp - 1, avp[j], et[j])
                                if nf < len(fillers):
                                    fillers[nf]()
                                    nf += 1
                    for j in range(2):
                        av(pt, qb, j, ncp - 1, avp[j], et[j])
                    while nf < len(fillers):
                        fillers[nf]()
                        nf += 1
                    # drain unnormalized AV + denominator row
                    for j in range(2):
                        nc.vector.tensor_copy(
                            out=o_un[pt, j][:, qb * QB:(qb + 1) * QB],
                            in_=avp[j][:])
                        norm_chain_pre(pt, j, qb)
                    # trailing qpT projection blocks overlap attention
                    if pt == 0 and qb + 1 < NQB:
                        proj_q(qb + 1)

            # ---- tail: last block's pt1 normalization, then all fc ----
            for j in range(2):
                norm_mult(1, j, NQB - 1)
            for sqc in range(S // 128):
                fc(sqc)

    if split_waits:
        _split_excess_waits(nc)
    return nc


def _prep_inputs(q, k, v, mask, W_qkv, b_qkv, W_out, b_out):
    """Host-side shard/layout prep. Returns (skv, in_maps)."""
    q = np.asarray(q, dtype=np.float32)
    k = np.asarray(k, dtype=np.float32)
    v = np.asarray(v, dtype=np.float32)
    mask = np.asarray(mask)
    W_qkv = np.asarray(W_qkv, dtype=np.float32)
    b_qkv = np.asarray(b_qkv, dtype=np.float32)
    W_out = np.asarray(W_out, dtype=np.float32)

    valid = [np.nonzero(mask[b, 0, 0] != 0)[0] for b in range(B)]
    cnts = [len(vi) for vi in valid]
    skv = max(512, max((c + 255) // 256 * 256 for c in cnts))
    nsk = skv // 128

    def tile_x(xT, inner):
        # [E, S'] -> [128, S'/ (NB*inner) ... ] partition-major contiguous
        n_out = xT.shape[1] // inner
        return np.ascontiguousarray(
            xT.reshape(NB, 128, n_out, inner).transpose(1, 2, 0, 3)
        ).astype(np.float16)

    qT, kc_t, vc_t, vms = [], [], [], []
    for b in range(B):
        qT.append(tile_x(np.ascontiguousarray(q[b].T), QB))
        kt = np.zeros((E, skv), np.float32)
        vt = np.zeros((E, skv), np.float32)
        kt[:, :cnts[b]] = k[b][valid[b]].T
        vt[:, :cnts[b]] = v[b][valid[b]].T
        # xk: [128, nkb, NB, 512] padded 512-blocks, contiguous per (p, kb)
        nkb = (skv + 511) // 512
        ktp = np.zeros((E, nkb * 512), np.float32)
        ktp[:, :skv] = kt
        kc_t.append(np.ascontiguousarray(
            ktp.reshape(NB, 128, nkb, 512).transpose(1, 2, 0, 3)
        ).astype(np.float16))
        vc_t.append(tile_x(vt, 128))
        vm = np.zeros((skv,), np.float32)
        vm[:cnts[b]] = 1.0
        vms.append(np.ascontiguousarray(vm.reshape(nsk, 128).T))

    def tile_w(wT):
        # [E, DC] -> [128, NB, DC]
        return np.ascontiguousarray(
            wT.reshape(NB, 128, DC).transpose(1, 0, 2)).astype(np.float16)

    in_maps = []
    for c in range(NCORES):
        b, g = divmod(c, GROUPS)
        sl = slice(g * DC, (g + 1) * DC)
        in_maps.append({
            "xq": qT[b], "xk": kc_t[b], "xv": vc_t[b],
            "wq": tile_w(np.ascontiguousarray(W_qkv[sl, :].T)),
            "wk": tile_w(np.ascontiguousarray(W_qkv[E:][sl, :].T)),
            "wv": tile_w(np.ascontiguousarray(W_qkv[2 * E:][sl, :].T)),
            "wo": np.ascontiguousarray(
                W_out[:, sl].T.reshape(2, 128, E).transpose(1, 0, 2)
            ).astype(np.float16),
            "bq": np.ascontiguousarray(b_qkv[sl].reshape(2, 128).T),
            "bk": np.ascontiguousarray(b_qkv[E:][sl].reshape(2, 128).T),
            "bv": np.ascontiguousarray(b_qkv[2 * E:][sl]),
            "vmask": vms[b],
            "ones64": np.ones((1, 64)).astype(ml_dtypes.bfloat16),
        })
    return skv, in_maps


def kernel(q, k, v, mask, W_qkv, b_qkv, W_out, b_out):
    from concourse import bass_utils

    skv, in_maps = _prep_inputs(q, k, v, mask, W_qkv, b_qkv, W_out, b_out)
    if skv not in _CACHE:
        _CACHE[skv] = _build(skv)
    nc = _CACHE[skv]

    trace = os.environ.get("KERNEL_TRACE") == "1"
    if trace:
        bass_utils.upload_artifacts = lambda tmpdir: "local://" + tmpdir
    res = bass_utils.run_bass_kernel_spmd(
        nc, in_maps, list(range(NCORES)), trace=trace)
    if trace:
        print(f"HW exec time: {res.exec_time_ns} ns")

    b_out = np.asarray(b_out, dtype=np.float32)
    out = np.zeros((B, S, E), np.float32)
    for c in range(NCORES):
        out[c // GROUPS] += res.results[c]["out"].astype(np.float32)
    out += b_out[None, None, :]
    return out


# revision 21
# speedup vs baseline: 1.1034x; 1.0353x over previous
"""Multi-head attention (B=2, S=2048, E=1024, H=16) on 8 TRN2 NeuronCores.

Sharding: batch x head-group. Core c handles batch b=c//4 and head group
g=c%4 (4 heads = 256 of E). Each core computes its heads' attention output
slice and a partial fc_out product [S, E]; the host sums the 4 partials per
batch and adds b_out.

v2: all-f16/bf16 matmul datapath (f16 runs ~2x f32r per-row on HW), whole
inputs preloaded with host-pretiled contiguous layouts, single fc pass with
K=256 PSUM accumulation (one [S,E] f16 output per core), gpsimd fc drains.

Device-side math per core (pairs pt in {0,1} of 2 heads = 128 dims):
  qpT = (Wq_g @ q[b].T + bq)      [256, S]   f16 (dims on partitions)
  kpT = (Wk_g @ k_c[b].T + bk)    [256, SKV] f16 (k compressed by mask)
  vp  = (v_c[b] @ Wv_g.T + bv)*m  [SKV, 4*65] bf16 (per head: 64 dims + mask col)
  S_T = kpT_h.T-chunks @ qpT_h    [SKV, S] per head (2 heads via tile_position)
  E_T = exp(S_T)  bf16            (no max-subtraction: |energy| <~ 45, safe)
  AV  = vp_aug.T @ E_T -> [65, S]: rows 0-63 unnormalized O_T, row 64 = denom
  O   = AV[0:64] * recip(AV[64])  (recip via [128,16] shuffle + ones-col
                                   broadcast matmul)
  out = O_T.T @ Wo (K=256 accumulated over both pairs in PSUM) -> [S, E] f16

Mask handling is exact: masked K/V rows are removed on the host (gather), so
softmax(where(mask==0, -1e20, e)) == exp(e_valid)/sum(exp(e_valid)).
"""

import os

import ml_dtypes
import numpy as np

B, S, E, H = 2, 2048, 1024, 16
D = E // H           # 64
NCORES = 8
GROUPS = 4           # head groups per batch (cores per batch)
HPG = H // GROUPS    # 4 heads per core
DC = E // GROUPS     # 256 dims per core
NB = E // 128        # 8 contraction chunks over E
QB = 512             # query block
NQB = S // QB        # 4

_CACHE = {}


def _split_excess_waits(nc, max_waits=1):
    """walrus rejects instructions carrying >1 sem wait; spread extras onto
    single-wait NoOps inserted before the instruction on the same engine."""
    import concourse.mybir as mybir

    n_split = 0
    for f in nc.m.functions:
        for bb in f.blocks:
            out, changed = [], False
            for ins in bb.instructions:
                si = ins.sync_info
                if si is not None and si.on_wait is not None and len(si.on_wait) > max_waits:
                    waits = list(si.on_wait)
                    for w in waits[:-max_waits]:
                        out.append(mybir.InstNoOp(
                            name=nc.get_next_instruction_name(),
                            engine=ins.engine, ins=[], outs=[],
                            sync_info=mybir.SyncInfo(on_wait=[w], on_update=[])))
                        n_split += 1
                    ins.sync_info = mybir.SyncInfo(
                        on_wait=waits[-max_waits:], on_update=list(si.on_update))
                    changed = True
                out.append(ins)
            if changed:
                bb.instructions = out
    return n_split


def _build(skv, split_waits=True):
    import concourse.bass as bass
    import concourse.mybir as mybir
    import concourse.tile as tile

    f32 = mybir.dt.float32
    f32r = mybir.dt.float32r
    f16 = mybir.dt.float16
    bf16 = mybir.dt.bfloat16
    Alu = mybir.AluOpType
    Act = mybir.ActivationFunctionType

    nsk = skv // 128     # 128-wide kv chunks
    ncp = nsk // 2       # exp chunk pairs (1024-wide exp tiles)
    kblocks = [512] * (skv // 512) + ([skv % 512] if skv % 512 else [])
    nkb = len(kblocks)   # xk stored as nkb padded 512-blocks

    nc = bass.Bass()
    xq_d = nc.declare_dram_parameter("xq", [128, NQB, NB, QB], f16, isOutput=False)
    xk_d = nc.declare_dram_parameter("xk", [128, nkb, NB, 512], f16, isOutput=False)
    xv_d = nc.declare_dram_parameter("xv", [128, nsk, NB, 128], f16, isOutput=False)
    wq_d = nc.declare_dram_parameter("wq", [128, NB, DC], f16, isOutput=False)
    wk_d = nc.declare_dram_parameter("wk", [128, NB, DC], f16, isOutput=False)
    wv_d = nc.declare_dram_parameter("wv", [128, NB, DC], f16, isOutput=False)
    wo_d = nc.declare_dram_parameter("wo", [128, 2, E], f16, isOutput=False)
    bq_d = nc.declare_dram_parameter("bq", [128, 2], f32, isOutput=False)
    bk_d = nc.declare_dram_parameter("bk", [128, 2], f32, isOutput=False)
    bv_d = nc.declare_dram_parameter("bv", [DC], f32, isOutput=False)
    vm_d = nc.declare_dram_parameter("vmask", [128, nsk], f32, isOutput=False)
    ones_d = nc.declare_dram_parameter("ones64", [1, 64], bf16, isOutput=False)
    out_d = nc.declare_dram_parameter("out", [S, E], f16, isOutput=True)

    with tile.TileContext(nc) as tc:
        with (
            tc.tile_pool(name="weights", bufs=1) as wpool,
            tc.tile_pool(name="xin", bufs=1) as xpool,
            tc.tile_pool(name="consts", bufs=1) as cpool,
            tc.tile_pool(name="persist", bufs=1) as ppool,
            tc.tile_pool(name="small", bufs=2) as smpool,
            tc.tile_pool(name="et", bufs=2) as etpool,
            tc.tile_pool(name="oun", bufs=4) as opool,
            tc.tile_pool(name="nrm", bufs=4) as nrpool,
            tc.tile_pool(name="outp", bufs=4) as obpool,
            tc.tile_pool(name="mm_ps", bufs=3, space="PSUM") as mmps,
            tc.tile_pool(name="av_ps", bufs=2, space="PSUM") as avps,
        ):
            # ---- weights / x / constants (k path first: it gates scores) ----
            wk_t = wpool.tile([128, NB, DC], f16, tag="wk")
            wq_t = wpool.tile([128, NB, DC], f16, tag="wq")
            wv_t = wpool.tile([128, NB, DC], f16, tag="wv")
            wo_t = wpool.tile([128, 2, E], f16, tag="wo")
            xq_t = xpool.tile([128, NQB, NB, QB], f16, tag="xq")
            xk_t = xpool.tile([128, nkb, NB, 512], f16, tag="xk")
            xv_t = xpool.tile([128, nsk, NB, 128], f16, tag="xv")
            bq_t = cpool.tile([128, 2], f32, tag="bq")
            bk_t = cpool.tile([128, 2], f32, tag="bk")
            bv_t = cpool.tile([128, DC], f32, tag="bv")
            vm_t = cpool.tile([128, nsk], f32, tag="vm")
            ones_t = cpool.tile([1, 64], bf16, tag="ones")

            def flat(ap):
                return ap.rearrange("p ... -> p (...)")

            nc.sync.dma_start(flat(wk_t[:]), flat(wk_d[:]))
            nc.sync.dma_start(bk_t[:], bk_d[:])
            for kb in range(nkb):
                nc.sync.dma_start(flat(xk_t[:, kb]), flat(xk_d[:, kb]))
            nc.sync.dma_start(flat(wq_t[:]), flat(wq_d[:]))
            nc.sync.dma_start(bq_t[:], bq_d[:])
            nc.sync.dma_start(flat(xq_t[:, 0]), flat(xq_d[:, 0]))
            nc.sync.dma_start(flat(wv_t[:]), flat(wv_d[:]))
            nc.sync.dma_start(bv_t[:], bv_d[None, :].to_broadcast((128, DC)))
            nc.sync.dma_start(vm_t[:], vm_d[:])
            nc.sync.dma_start(ones_t[:], ones_d[:])
            for h in range(2):
                nc.sync.dma_start(flat(xv_t[:, h * (nsk // 2):(h + 1) * (nsk // 2)]),
                                  flat(xv_d[:, h * (nsk // 2):(h + 1) * (nsk // 2)]))
            for nb in range(1, NQB):
                nc.sync.dma_start(flat(xq_t[:, nb]), flat(xq_d[:, nb]))
            nc.sync.dma_start(flat(wo_t[:]), flat(wo_d[:]))

            # ---- persistent activations ----
            qpT = ppool.tile([128, 2, S], f16, tag="qpT")
            kpT = ppool.tile([128, 2, skv], f16, tag="kpT")
            vp = ppool.tile([128, nsk, HPG * (D + 1)], bf16, tag="vp")
            o_f16 = ppool.tile([128, 2, S], f16, tag="o_f16")

            def proj_k(kb, w):
                for mc in range(2):
                    ps = mmps.tile([128, 1024], f32, tag="mm", name="kp_ps")
                    for kc in range(NB):
                        nc.tensor.matmul(
                            ps[:, 0:w], wk_t[:, kc, mc * 128:(mc + 1) * 128],
                            xk_t[:, kb, kc, 0:w],
                            start=(kc == 0), stop=(kc == NB - 1))
                    nc.vector.tensor_tensor(
                        out=kpT[:, mc, kb * 512:kb * 512 + w], in0=ps[:, 0:w],
                        in1=bk_t[:, mc:mc + 1].to_broadcast((128, w)), op=Alu.add)

            def proj_q(nb):
                for mc in range(2):
                    ps = mmps.tile([128, 1024], f32, tag="mm", name="qp_ps")
                    for kc in range(NB):
                        nc.tensor.matmul(
                            ps[:, 0:512], wq_t[:, kc, mc * 128:(mc + 1) * 128],
                            xq_t[:, nb, kc, :], start=(kc == 0), stop=(kc == NB - 1))
                    nc.vector.tensor_tensor(
                        out=qpT[:, mc, nb * 512:(nb + 1) * 512], in0=ps[:, 0:512],
                        in1=bq_t[:, mc:mc + 1].to_broadcast((128, 512)), op=Alu.add)

            def proj_v(sc):
                ps = mmps.tile([128, 1024], f32, tag="mm", name="vp_ps")
                for kc in range(NB):
                    nc.tensor.matmul(
                        ps[:, 0:DC], xv_t[:, sc, kc, :], wv_t[:, kc, :],
                        start=(kc == 0), stop=(kc == NB - 1))
                t1 = smpool.tile([128, DC], f32, tag="vtmp")
                nc.vector.tensor_tensor(out=t1[:], in0=ps[:, 0:DC], in1=bv_t[:],
                                        op=Alu.add)
                vps = vp[:, sc, :].rearrange("p (h w) -> p h w", w=D + 1)
                nc.vector.tensor_tensor(
                    out=vps[:, :, 0:D],
                    in0=t1.rearrange("p (h w) -> p h w", w=D),
                    in1=vm_t[:, sc:sc + 1, None].to_broadcast((128, HPG, D)),
                    op=Alu.mult)
                nc.vector.tensor_copy(
                    out=vps[:, :, D:D + 1],
                    in_=vm_t[:, sc:sc + 1, None].to_broadcast((128, HPG, 1)))

            # lead-in: kpT fully, first qpT block, first two vp chunks
            for kb, w in enumerate(kblocks):
                proj_k(kb, w)
            proj_q(0)
            proj_v(0)
            proj_v(1)

            o_un = {}    # (pt, j) -> [65, S] bf16 unnormalized O_T + denom row
            rcr = {}     # (pt, j) -> [1, S] f16 reciprocal denominators

            def scores(pt, qb, j, cp):
                """2 score matmuls (skc=2cp, 2cp+1) + 1024-wide exp."""
                psx = mmps.tile([128, 1024], f32, tag="mm", name=f"sc{j}")
                for h in range(2):
                    skc = 2 * cp + h
                    nc.tensor.matmul(
                        psx[:, h * 512:(h + 1) * 512],
                        kpT[64 * j:64 * j + 64, pt, skc * 128:(skc + 1) * 128],
                        qpT[64 * j:64 * j + 64, pt, qb * QB:(qb + 1) * QB],
                        start=True, stop=True, tile_position=(64 * j, 0))
                return psx

            def expv(psx, et, cp):
                nc.scalar.activation(
                    et[:, 2 * cp:2 * cp + 2, :].rearrange("p a b -> p (a b)"),
                    psx[:], Act.Exp)

            def av(pt, qb, j, cp, av_ps, et):
                hl = 2 * pt + j
                for h in range(2):
                    skc = 2 * cp + h
                    nc.tensor.matmul(
                        av_ps[:], vp[:, skc, hl * (D + 1):(hl + 1) * (D + 1)],
                        et[:, skc, :], start=(skc == 0), stop=(skc == nsk - 1))

            def norm_chain_pre(pt, j, qb):
                """recip of one qb's denominators: [1,QB] -> [128,4] shuffle,
                recip, cast bf16, shuffle back. Latency hides under compute."""
                s128 = nrpool.tile([128, QB // 128], bf16, tag="s128")
                nc.sync.dma_start(s128[:], o_un[pt, j][64:65, qb * QB:(qb + 1) * QB])
                r32 = nrpool.tile([128, QB // 128], f32, tag="r32")
                nc.vector.reciprocal(out=r32[:], in_=s128[:])
                rr = nrpool.tile([128, QB // 128], bf16, tag="r16")
                nc.vector.tensor_copy(out=rr[:], in_=r32[:])
                nc.sync.dma_start(rcr[pt, j][0:1, qb * QB:(qb + 1) * QB], rr[:])

            def norm_mult(pt, j, qb):
                """o_f16 = o_un * bcast(recip) for one qb slice (PE+DVE)."""
                rc_ps = mmps.tile([128, 1024], f32, tag="mm", name="rc_ps")
                nc.tensor.matmul(
                    rc_ps[0:64, 0:512], ones_t[:],
                    rcr[pt, j][0:1, qb * QB:(qb + 1) * QB], start=True, stop=True)
                nc.vector.tensor_tensor(
                    out=o_f16[64 * j:64 * j + 64, pt, qb * QB:(qb + 1) * QB],
                    in0=o_un[pt, j][0:64, qb * QB:(qb + 1) * QB],
                    in1=rc_ps[0:64, 0:512], op=Alu.mult)

            def fc(sqc, drain="alt"):
                ps = mmps.tile([128, 1024], f32, tag="mm", name="fc")
                for eb in range(2):
                    for pt in range(2):
                        nc.tensor.matmul(
                            ps[:, eb * 512:(eb + 1) * 512],
                            o_f16[:, pt, sqc * 128:(sqc + 1) * 128],
                            wo_t[:, pt, eb * 512:(eb + 1) * 512],
                            start=(pt == 0), stop=(pt == 1))
                ob = obpool.tile([128, 1024], f16, tag="ob")
                if drain == "v" or (drain == "alt" and sqc % 2 == 1):
                    nc.vector.tensor_copy(out=ob[:], in_=ps[:])
                else:
                    nc.scalar.activation(ob[:], ps[:], Act.Copy)
                nc.sync.dma_start(out_d[sqc * 128:(sqc + 1) * 128, :], ob[:])

            # ---- attention: pt pairs x query blocks, hand-interleaved ----
            for pt in range(2):
                for j in range(2):
                    o_un[pt, j] = opool.tile([65, S], bf16, tag="oun",
                                             name=f"o_un{pt}{j}")
                    rcr[pt, j] = nrpool.tile([1, S], bf16, tag="rcr",
                                             name=f"rcr{pt}{j}")
                for qb in range(NQB):
                    # fillers executed early in this qb's PE stream
                    fillers = []
                    if pt == 0 and qb == 0:
                        fillers = [lambda sc=sc: proj_v(sc) for sc in range(2, nsk)]
                    if pt == 1:
                        # pt0 normalization interleaves into pt1's blocks
                        fillers = [lambda j=j, qb=qb: norm_mult(0, j, qb)
                                   for j in range(2)]

                    et = {j: etpool.tile([128, nsk, 512], bf16, tag="et",
                                         name=f"et{j}") for j in range(2)}
                    avp = {j: avps.tile([65, 512], f32, tag="av", name=f"av{j}")
                           for j in range(2)}
                    nf = 0
                    for cp in range(ncp):
                        for j in range(2):
                            psx = scores(pt, qb, j, cp)
                            expv(psx, et[j], cp)
                            if cp > 0:
                                av(pt, qb, j, cp - 1, avp[j], et[j])
                                if nf < len(fillers):
                                    fillers[nf]()
                                    nf += 1
                    for j in range(2):
                        av(pt, qb, j, ncp - 1, avp[j], et[j])
                    while nf < len(fillers):
                        fillers[nf]()
                        nf += 1
                    # drain unnormalized AV + denominator row
                    for j in range(2):
                        nc.vector.tensor_copy(
                            out=o_un[pt, j][:, qb * QB:(qb + 1) * QB],
                            in_=avp[j][:])
                        norm_chain_pre(pt, j, qb)
                    # trailing qpT projection blocks overlap attention
                    if pt == 0 and qb + 1 < NQB:
                        proj_q(qb + 1)

            # ---- tail: pt1 normalization + fused fc over both pairs ----
            for fqb in range(NQB):
                for j in range(2):
                    norm_mult(1, j, fqb)
                for sqc in range(fqb * 4, (fqb + 1) * 4):
                    fc(sqc)

    if split_waits:
        _split_excess_waits(nc)
    return nc


def _prep_inputs(q, k, v, mask, W_qkv, b_qkv, W_out, b_out):
    """Host-side shard/layout prep. Returns (skv, in_maps)."""
    q = np.asarray(q, dtype=np.float32)
    k = np.asarray(k, dtype=np.float32)
    v = np.asarray(v, dtype=np.float32)
    mask = np.asarray(mask)
    W_qkv = np.asarray(W_qkv, dtype=np.float32)
    b_qkv = np.asarray(b_qkv, dtype=np.float32)
    W_out = np.asarray(W_out, dtype=np.float32)

    valid = [np.nonzero(mask[b, 0, 0] != 0)[0] for b in range(B)]
    cnts = [len(vi) for vi in valid]
    skv = max(512, max((c + 255) // 256 * 256 for c in cnts))
    nsk = skv // 128

    def tile_x(xT, inner):
        # [E, S'] -> [128, S'/ (NB*inner) ... ] partition-major contiguous
        n_out = xT.shape[1] // inner
        return np.ascontiguousarray(
            xT.reshape(NB, 128, n_out, inner).transpose(1, 2, 0, 3)
        ).astype(np.float16)

    qT, kc_t, vc_t, vms = [], [], [], []
    for b in range(B):
        qT.append(tile_x(np.ascontiguousarray(q[b].T), QB))
        kt = np.zeros((E, skv), np.float32)
        vt = np.zeros((E, skv), np.float32)
        kt[:, :cnts[b]] = k[b][valid[b]].T
        vt[:, :cnts[b]] = v[b][valid[b]].T
        # xk: [128, nkb, NB, 512] padded 512-blocks, contiguous per (p, kb)
        nkb = (skv + 511) // 512
        ktp = np.zeros((E, nkb * 512), np.float32)
        ktp[:, :skv] = kt
        kc_t.append(np.ascontiguousarray(
            ktp.reshape(NB, 128, nkb, 512).transpose(1, 2, 0, 3)
        ).astype(np.float16))
        vc_t.append(tile_x(vt, 128))
        vm = np.zeros((skv,), np.float32)
        vm[:cnts[b]] = 1.0
        vms.append(np.ascontiguousarray(vm.reshape(nsk, 128).T))

    def tile_w(wT):
        # [E, DC] -> [128, NB, DC]
        return np.ascontiguousarray(
            wT.reshape(NB, 128, DC).transpose(1, 0, 2)).astype(np.float16)

    in_maps = []
    for c in range(NCORES):
        b, g = divmod(c, GROUPS)
        sl = slice(g * DC, (g + 1) * DC)
        in_maps.append({
            "xq": qT[b], "xk": kc_t[b], "xv": vc_t[b],
            "wq": tile_w(np.ascontiguousarray(W_qkv[sl, :].T)),
            "wk": tile_w(np.ascontiguousarray(W_qkv[E:][sl, :].T)),
            "wv": tile_w(np.ascontiguousarray(W_qkv[2 * E:][sl, :].T)),
            "wo": np.ascontiguousarray(
                W_out[:, sl].T.reshape(2, 128, E).transpose(1, 0, 2)
            ).astype(np.float16),
            "bq": np.ascontiguousarray(b_qkv[sl].reshape(2, 128).T),
            "bk": np.ascontiguousarray(b_qkv[E:][sl].reshape(2, 128).T),
            "bv": np.ascontiguousarray(b_qkv[2 * E:][sl]),
            "vmask": vms[b],
            "ones64": np.ones((1, 64)).astype(ml_dtypes.bfloat16),
        })
    return skv, in_maps


def kernel(q, k, v, mask, W_qkv, b_qkv, W_out, b_out):
    from concourse import bass_utils

    skv, in_maps = _prep_inputs(q, k, v, mask, W_qkv, b_qkv, W_out, b_out)
    if skv not in _CACHE:
        _CACHE[skv] = _build(skv)
    nc = _CACHE[skv]

    trace = os.environ.get("KERNEL_TRACE") == "1"
    if trace:
        bass_utils.upload_artifacts = lambda tmpdir: "local://" + tmpdir
    res = bass_utils.run_bass_kernel_spmd(
        nc, in_maps, list(range(NCORES)), trace=trace)
    if trace:
        print(f"HW exec time: {res.exec_time_ns} ns")

    b_out = np.asarray(b_out, dtype=np.float32)
    out = np.zeros((B, S, E), np.float32)
    for c in range(NCORES):
        out[c // GROUPS] += res.results[c]["out"].astype(np.float32)
    out += b_out[None, None, :]
    return out
